# revision 4
# baseline (speedup 1.0000x reference)
"""Channel-group winner-take-all (group size 4) on 8 TRN2 NeuronCores.

Full input x: [32, 512, 56, 56] f32. Within each contiguous group of 4
channels, keep elements equal to the group max, zero the rest.

Sharding: data parallel over batch — each of the 8 cores handles 4 batches.
Per-core layout: partition dim = 128 channel groups, free dim = (member,
spatial chunk). All compute on the Vector engine; DMA via HWDGE (nc.sync).
"""

import sys

for _p in ("/opt/trn_rl_repo",):
    if _p not in sys.path:
        sys.path.insert(0, _p)

import numpy as np
import concourse.bacc as bacc
import concourse.bass as bass
import concourse.mybir as mybir
from concourse.tile import TileContext
from concourse.bass_utils import run_bass_kernel_spmd

N_CORES = 8
B, C, H, W = 32, 512, 56, 56
S = H * W  # 3136
M = 4  # channel group size
G = C // M  # 128 groups == SBUF partition count
B_PER_CORE = B // N_CORES  # 4
SCHUNK = 784  # spatial chunk per tile; 3136 = 4 * 784
N_CHUNKS = S // SCHUNK


def build_nc(compile=True):
    nc = bacc.Bacc()
    x = nc.declare_dram_parameter(
        "x", [B_PER_CORE, C, S], mybir.dt.float32, isOutput=False
    )
    out = nc.declare_dram_parameter(
        "out", [B_PER_CORE, C, S], mybir.dt.float32, isOutput=True
    )
    xv = x.rearrange("b (g m) s -> b g m s", m=M)
    ov = out.rearrange("b (g m) s -> b g m s", m=M)

    with TileContext(nc) as tc:
        with tc.tile_pool(name="io", bufs=3) as io_pool, tc.tile_pool(
            name="tmp", bufs=3
        ) as tmp_pool:
            for b in range(B_PER_CORE):
                for j in range(N_CHUNKS):
                    sl = slice(j * SCHUNK, (j + 1) * SCHUNK)
                    xt = io_pool.tile([G, M, SCHUNK], mybir.dt.float32, tag="x")
                    ot = io_pool.tile([G, M, SCHUNK], mybir.dt.float32, tag="o")
                    pm = tmp_pool.tile([G, 2, SCHUNK], mybir.dt.float32, tag="pm")

                    nc.sync.dma_start(out=xt[:], in_=xv[b, :, :, sl])

                    # pairwise max of members (0,1) and (2,3) in one strided op
                    xp = xt[:].rearrange("p (a two) s -> p a two s", two=2)
                    nc.vector.tensor_tensor(
                        pm[:], xp[:, :, 0, :], xp[:, :, 1, :], mybir.AluOpType.max
                    )
                    # group max into pm[:, 0, :]
                    nc.vector.tensor_max(pm[:, 0, :], pm[:, 0, :], pm[:, 1, :])
                    # mask = (x >= gmax) via step-0 broadcast of gmax over members
                    gb = pm[:, 0:1, :].broadcast_to((G, M, SCHUNK))
                    nc.vector.tensor_tensor(
                        ot[:], xt[:], gb, mybir.AluOpType.is_ge
                    )
                    # out = mask * x
                    nc.vector.tensor_mul(ot[:], ot[:], xt[:])

                    nc.sync.dma_start(out=ov[b, :, :, sl], in_=ot[:])
    if compile:
        nc.compile()
    return nc


_NC = None


def get_nc():
    global _NC
    if _NC is None:
        _NC = build_nc()
    return _NC


def make_in_maps(x):
    """x: [B, C, S] f32 contiguous -> per-core input maps."""
    return [
        {"x": x[i * B_PER_CORE : (i + 1) * B_PER_CORE]} for i in range(N_CORES)
    ]


def kernel(x):
    x = np.ascontiguousarray(np.asarray(x, dtype=np.float32)).reshape(B, C, S)
    nc = get_nc()
    res = run_bass_kernel_spmd(nc, make_in_maps(x), core_ids=list(range(N_CORES)))
    out = np.concatenate(
        [res.results[i]["out"].reshape(B_PER_CORE, C, S) for i in range(N_CORES)],
        axis=0,
    )
    return out.reshape(B, C, H, W)
